# revision 1
# baseline (speedup 1.0000x reference)
"""MultiHeadAttention Trainium2 Bass kernel (8 NeuronCores).

Reference computes (per batch b):
  qp = q @ Wq.T + bq            [S, H*D]   (S=2048, H=8, D=256)
  q_h = qp.reshape(H, S, D)     -- RAW reshape, not split-heads:
        head h <- qp rows [h*256,(h+1)*256), all 2048 cols;
        within head: s2 = ls*8 + g , d  <-> qp[h*256+ls, g*256+d]
  scores_h = q_h @ k_h.T / 16 ; P = softmax ; o_h = P @ v_h
  out[s2, h*256+d] = o_h[s2, d] ;  y = out @ Wo.T + bo

Sharding: core c = (b = c//2, hg = c%2) handles batch b, heads
hg*4..hg*4+4. Head h only needs x rows [h*256,(h+1)*256) -> each core
gets a [256(d), 1024(s)] transposed slice of q/k/v. Within a head we
work in the permuted sequence order s2' = g*256 + ls (softmax is
row-wise so a consistent permutation of rows/cols is exact); the
inverse permutation is applied by the final strided DMA to DRAM.

Everything runs in f32r matmuls (TF32-like, ~1.5e-4 rel err) with fp32
accumulation. Scores are produced transposed ([key-chunk partitions x
query free]); the softmax denominator is an ones-vector matmul; the
reciprocal is broadcast across partitions on the idle GpSimd engine.

Emission is software-pipelined: output-projection matmuls for group
ig-1 are emitted after the QK matmuls of group ig, and head h+1's
projections before head h's last output projection, so the PE never
waits on the (DVE/GpSimd) normalize chain.

Host: transposes/slices inputs (zero device cost), sums the two
half-partials per batch, adds bo.
"""

import os as _os
import numpy as np

B, S, D, H = 4, 2048, 256, 8
HG = 2            # head groups (cores per batch)
HPG = H // HG     # heads per group = 4
SH = S // H       # seq rows owned by one head = 256
NCORES = 8
SCALE = 1.0 / 16.0  # 1/sqrt(d_k)

_CACHE = {}
# PSUM pool sizing (8 banks total): tuned on HW — A2 + S4 + O2.
BUFS_A = 2   # proj psum + rowsum accumulator
BUFS_S = 4   # score tiles (QK -> exp pipeline depth)
BUFS_O = 2   # PV accumulator pair / outproj psum
BUFS_P = 16  # probability tiles in SBUF (one full i'-group)


def _build():
    import concourse.bacc as bacc
    import concourse.mybir as mybir
    from concourse.tile import TileContext

    F32 = mybir.dt.float32
    F32R = mybir.dt.float32r
    EXP = mybir.ActivationFunctionType.Exp
    MULT = mybir.AluOpType.mult
    ADD = mybir.AluOpType.add

    nc = bacc.Bacc("TRN2", target_bir_lowering=False)

    # ---- DRAM I/O (per-core SPMD) ----
    xqT_d = nc.dram_tensor("xqT", [D, HPG * SH], F32R, kind="ExternalInput")
    xkT_d = nc.dram_tensor("xkT", [D, HPG * SH], F32R, kind="ExternalInput")
    xvT_d = nc.dram_tensor("xvT", [D, HPG * SH], F32R, kind="ExternalInput")
    WqT_d = nc.dram_tensor("WqT", [D, S], F32R, kind="ExternalInput")
    WkT_d = nc.dram_tensor("WkT", [D, S], F32R, kind="ExternalInput")
    WvT_d = nc.dram_tensor("WvT", [D, S], F32R, kind="ExternalInput")
    WoT_d = nc.dram_tensor("WoT", [HPG * D, D], F32R, kind="ExternalInput")
    bqT_d = nc.dram_tensor("bqT", [128, 16], F32, kind="ExternalInput")
    bkT_d = nc.dram_tensor("bkT", [128, 16], F32, kind="ExternalInput")
    bvr_d = nc.dram_tensor("bvr", [1, S], F32, kind="ExternalInput")
    out_d = nc.dram_tensor("part", [S, D], F32, kind="ExternalOutput")

    with TileContext(nc) as tc:
        with nc.allow_low_precision(reason="f32r matmul rounding"), \
             tc.tile_pool(name="sb", bufs=1) as sb, \
             tc.tile_pool(name="ps", bufs=1, space="PSUM") as ps:

            def sbt(shape, dt, tag, **kw):
                return sb.tile(shape, dt, tag=tag, name=tag, **kw)

            # ---- persistent SBUF tiles ----
            WqT = [sbt([128, S], F32R, f"wq{i}") for i in range(2)]
            WkT = [sbt([128, S], F32R, f"wk{i}") for i in range(2)]
            WvT = [sbt([128, S], F32R, f"wv{i}") for i in range(2)]
            xqT = [sbt([128, HPG * SH], F32R, f"xq{i}") for i in range(2)]
            xkT = [sbt([128, HPG * SH], F32R, f"xk{i}") for i in range(2)]
            xvT = [sbt([128, HPG * SH], F32R, f"xv{i}") for i in range(2)]
            WoT = [sbt([128, D], F32R, f"wo{i}") for i in range(8)]
            bqT = sbt([128, 16], F32, "bqT")
            bkT = sbt([128, 16], F32, "bkT")
            bvr = sbt([1, S], F32, "bvr")
            bvb = sbt([128, S], F32, "bvb")  # bv broadcast across partitions

            # startup-critical DMAs first, split + interleaved so the
            # earliest Q-proj matmuls can start after ~1MB has landed;
            # spread across both HWDGE queues (sync: weights, scalar: x).
            nc.scalar.dma_start(bqT[:], bqT_d[:])
            for i in range(2):
                nc.scalar.dma_start(xqT[i][:, 0:512], xqT_d[i * 128:(i + 1) * 128, 0:512])
            for q in range(4):
                for i in range(2):
                    nc.sync.dma_start(WqT[i][:, q * 512:(q + 1) * 512],
                                      WqT_d[i * 128:(i + 1) * 128,
                                            q * 512:(q + 1) * 512])
            nc.scalar.dma_start(bkT[:], bkT_d[:])
            for i in range(2):
                nc.scalar.dma_start(xkT[i][:, 0:512], xkT_d[i * 128:(i + 1) * 128, 0:512])
            for q in range(4):
                for i in range(2):
                    nc.sync.dma_start(WkT[i][:, q * 512:(q + 1) * 512],
                                      WkT_d[i * 128:(i + 1) * 128,
                                            q * 512:(q + 1) * 512])
            nc.scalar.dma_start(bvr[:], bvr_d[:])
            for i in range(2):
                nc.scalar.dma_start(xvT[i][:, 0:512], xvT_d[i * 128:(i + 1) * 128, 0:512])
            for q in range(4):
                for i in range(2):
                    nc.sync.dma_start(WvT[i][:, q * 512:(q + 1) * 512],
                                      WvT_d[i * 128:(i + 1) * 128,
                                            q * 512:(q + 1) * 512])
            for i in range(2):
                nc.scalar.dma_start(xqT[i][:, 512:1024], xqT_d[i * 128:(i + 1) * 128, 512:1024])
                nc.scalar.dma_start(xkT[i][:, 512:1024], xkT_d[i * 128:(i + 1) * 128, 512:1024])
                nc.scalar.dma_start(xvT[i][:, 512:1024], xvT_d[i * 128:(i + 1) * 128, 512:1024])
            for i in range(8):
                nc.scalar.dma_start(WoT[i][:], WoT_d[i * 128:(i + 1) * 128, :])

            nc.gpsimd.partition_broadcast(bvb[:], bvr[:])

            ones_f = sbt([128, 1], F32, "ones_f")
            nc.vector.memset(ones_f[:], 1.0)
            ones_col = sbt([128, 1], F32R, "ones_col")
            nc.vector.tensor_copy(ones_col[:], ones_f[:])

            qprojT = [sbt([128, S], F32R, f"qprojT{i}") for i in range(2)]
            kprojT = [sbt([128, S], F32R, f"kprojT{i}") for i in range(2)]
            vproj = [sbt([128, S], F32R, f"vproj{i}") for i in range(2)]
            yacc = [sbt([128, D], F32, f"yacc{i}") for i in range(16)]

            NG = S // 512  # 4 i'-groups of 512

            def emit_proj(lh, split_qk=False, mid_hook=None, do_v=True):
                """Q/K/V projections for head lh into qprojT/kprojT/vproj.
                split_qk: emit all Q before all K (head 0: lets the PE
                start while the K/V DMAs are still streaming in)."""
                scol = lh * SH

                def q_chunk(ec):
                    g, dct = divmod(ec, 2)
                    pq = ps.tile([128, 512], F32, tag="A", bufs=BUFS_A, name="pq")
                    for dc in range(2):
                        nc.tensor.matmul(
                            pq[:, :SH],
                            WqT[dc][:, ec * 128:(ec + 1) * 128],
                            xqT[dc][:, scol:scol + SH],
                            start=(dc == 0), stop=(dc == 1))
                    nc.vector.tensor_scalar(
                        out=qprojT[dct][:, g * SH:(g + 1) * SH],
                        in0=pq[:, :SH], scalar1=SCALE,
                        scalar2=bqT[:, ec:ec + 1], op0=MULT, op1=ADD)

                def k_chunk(ec):
                    g, dct = divmod(ec, 2)
                    pk = ps.tile([128, 512], F32, tag="O", bufs=BUFS_O, name="pk")
                    for dc in range(2):
                        nc.tensor.matmul(
                            pk[:, :SH],
                            WkT[dc][:, ec * 128:(ec + 1) * 128],
                            xkT[dc][:, scol:scol + SH],
                            start=(dc == 0), stop=(dc == 1))
                    nc.vector.tensor_scalar(
                        out=kprojT[dct][:, g * SH:(g + 1) * SH],
                        in0=pk[:, :SH], scalar1=bkT[:, ec:ec + 1],
                        scalar2=None, op0=ADD)

                if split_qk:
                    for ec in range(16):
                        q_chunk(ec)
                    for ec in range(16):
                        k_chunk(ec)
                else:
                    for ec in range(4):
                        q_chunk(ec)
                    if mid_hook is not None:
                        mid_hook()
                    for ec in range(4, 16):
                        q_chunk(ec)
                        k_chunk(ec - 4)
                    if do_v:
                        # V before the K tail: its DVE adds aren't queued
                        # behind the K copies, and the K-tail copies drain
                        # during the next QK phase (their consumers run
                        # ~6us later, at jc>=12).
                        emit_vproj(lh)
                    for ec in range(12, 16):
                        k_chunk(ec)
                    return
                if not do_v:
                    return

                emit_vproj(lh)

            def emit_vproj(lh):
                scol = lh * SH
                for sc in range(2):
                    for ng in range(NG):
                        pv = ps.tile([128, 512], F32, tag="A", bufs=BUFS_A, name="pv")
                        for dc in range(2):
                            nc.tensor.matmul(
                                pv[:],
                                xvT[dc][:, scol + sc * 128:scol + (sc + 1) * 128],
                                WvT[dc][:, ng * 512:(ng + 1) * 512],
                                start=(dc == 0), stop=(dc == 1))
                        nc.vector.tensor_add(
                            vproj[sc][:, ng * 512:(ng + 1) * 512], pv[:],
                            bvb[:, ng * 512:(ng + 1) * 512])

            def emit_qk(lh, ig, p_tiles):
                icol = ig * 512
                for jc in range(16):
                    sp = ps.tile([128, 512], F32, tag="S", bufs=BUFS_S, name="sp")
                    for dc in range(2):
                        nc.tensor.matmul(
                            sp[:],
                            kprojT[dc][:, jc * 128:(jc + 1) * 128],
                            qprojT[dc][:, icol:icol + 512],
                            start=(dc == 0), stop=(dc == 1))
                    pt = sb.tile([128, 512], F32R, tag="p", bufs=BUFS_P, name="pt")
                    nc.scalar.activation(pt[:], sp[:], EXP)
                    p_tiles.append(pt)

            def emit_pv(lh, ig, p_tiles, state):
                rs = ps.tile([128, 512], F32, tag="A", bufs=BUFS_A, name="rs")
                o_ps = [ps.tile([128, 512], F32, tag="O", bufs=BUFS_O, name=f"o{dc}")
                        for dc in range(2)]
                for jc in range(16):
                    g, half = divmod(jc, 2)
                    nc.tensor.matmul(
                        rs[0:1, :], ones_col[:], p_tiles[jc][:],
                        start=(jc == 0), stop=(jc == 15),
                        skip_group_check=True)
                    for dc in range(2):
                        nc.tensor.matmul(
                            o_ps[dc][:],
                            vproj[half][:, g * SH + dc * 128:g * SH + (dc + 1) * 128],
                            p_tiles[jc][:],
                            start=(jc == 0), stop=(jc == 15),
                            skip_group_check=True)
                state[ig] = (rs, o_ps)

            def emit_norm(lh, ig, state):
                """recip -> gpsimd broadcast -> DVE muls (no PE)."""
                rs, o_ps = state[ig]
                rcp = sb.tile([1, 512], F32, tag="rcp", bufs=1, name="rcp")
                nc.vector.reciprocal(rcp[:], rs[0:1, :])
                bc_sb = sb.tile([128, 512], F32, tag="bc_sb", bufs=2,
                                name="bc_sb")
                nc.gpsimd.partition_broadcast(bc_sb[:], rcp[:])
                onrm = [sb.tile([128, 512], F32R, tag="onrm", bufs=4,
                                name=f"onrm{dc}") for dc in range(2)]
                for dc in range(2):
                    nc.vector.tensor_mul(onrm[dc][:], o_ps[dc][:], bc_sb[:])
                state[(ig, "onrm")] = onrm

            def emit_outproj(lh, ig, state, last_head):
                onrm = state[(ig, "onrm")]
                for sub in range(4):
                    yp = ps.tile([128, 512], F32, tag="O", bufs=BUFS_O, name="yp")
                    for dc in range(2):
                        nc.tensor.matmul(
                            yp[:, :D],
                            onrm[dc][:, sub * 128:(sub + 1) * 128],
                            WoT[lh * 2 + dc][:],
                            start=(dc == 0), stop=(dc == 1))
                    t = ig * 4 + sub
                    if lh == 0:
                        nc.vector.tensor_copy(yacc[t][:], yp[:, :D])
                    else:
                        nc.vector.tensor_add(yacc[t][:], yacc[t][:], yp[:, :D])
                    if last_head:
                        g, half = divmod(t, 2)
                        nc.sync.dma_start(
                            out_r[g, half * 128:(half + 1) * 128, :], yacc[t][:])

            out_r = out_d.rearrange("(ls g) o -> g ls o", g=8)

            emit_proj(0, split_qk=True)
            for lh in range(HPG):
                last = lh == HPG - 1
                state = {}
                for ig in range(NG):
                    p_tiles = []
                    emit_qk(lh, ig, p_tiles)
                    if ig >= 1:
                        emit_outproj(lh, ig - 1, state, last)
                    emit_pv(lh, ig, p_tiles, state)
                    emit_norm(lh, ig, state)
                if not last:
                    emit_proj(lh + 1,
                              mid_hook=lambda: emit_outproj(lh, NG - 1, state, last))
                else:
                    emit_outproj(lh, NG - 1, state, last)

    nc.finalize()
    return nc


def _get_nc():
    if "nc" not in _CACHE:
        _CACHE["nc"] = _build()
    return _CACHE["nc"]


def _prep_inputs(query, key, values, Wq, bq, Wk, bk, Wv, bv, Wo, bo):
    f32 = np.float32
    query = np.asarray(query, f32)
    key = np.asarray(key, f32)
    values = np.asarray(values, f32)
    WqT = np.ascontiguousarray(np.asarray(Wq, f32).T)
    WkT = np.ascontiguousarray(np.asarray(Wk, f32).T)
    WvT = np.ascontiguousarray(np.asarray(Wv, f32).T)
    WoT = np.ascontiguousarray(np.asarray(Wo, f32).T)
    bqT = np.ascontiguousarray((np.asarray(bq, f32) * SCALE).reshape(16, 128).T)
    bkT = np.ascontiguousarray(np.asarray(bk, f32).reshape(16, 128).T)
    bvr = np.ascontiguousarray(np.asarray(bv, f32).reshape(1, S))

    in_maps = []
    for c in range(NCORES):
        b, hg = divmod(c, HG)
        rows = slice(hg * HPG * SH, (hg + 1) * HPG * SH)
        in_maps.append({
            "xqT": np.ascontiguousarray(query[b, rows, :].T),
            "xkT": np.ascontiguousarray(key[b, rows, :].T),
            "xvT": np.ascontiguousarray(values[b, rows, :].T),
            "WqT": WqT, "WkT": WkT, "WvT": WvT,
            "WoT": np.ascontiguousarray(WoT[hg * HPG * D:(hg + 1) * HPG * D, :]),
            "bqT": bqT, "bkT": bkT, "bvr": bvr,
        })
    return in_maps


def _enable_tracing_shims():
    """Best-effort: make trace=True survivable in environments where the
    image's antenv lacks axon_hooks (registers the NTFF hook from the boot
    shim) and where artifact upload has no network (keep local)."""
    import sys
    import types
    try:
        import antenv.axon_hooks  # noqa: F401
    except Exception:
        try:
            from trn_agent_boot.trn_boot import _ntff_profile_via_ctypes
            hook = _ntff_profile_via_ctypes("/opt/axon/libaxon_pjrt.so")
            mod = types.ModuleType("antenv.axon_hooks")
            mod.get_axon_ntff_profile_hook = lambda: hook
            mod.set_axon_ntff_profile_hook = lambda h: None
            sys.modules["antenv.axon_hooks"] = mod
            import antenv
            antenv.axon_hooks = mod
        except Exception:
            pass
    try:
        import concourse.bass_utils as bu
        from concourse._compat import FishPath
        FishPath.bucket_root()  # raises when no bucket/network configured
    except Exception:
        try:
            bu.upload_artifacts = lambda tmpdir: f"local://{tmpdir}"
        except Exception:
            pass


def kernel(**inputs):
    import os
    from concourse.bass_utils import run_bass_kernel_spmd

    nc = _get_nc()
    in_maps = _prep_inputs(**inputs)
    trace = bool(int(os.environ.get("KERNEL_TRACE", "0")))
    if trace or os.environ.get("BASS_TRACE"):
        _enable_tracing_shims()
    res = run_bass_kernel_spmd(nc, in_maps, core_ids=list(range(NCORES)),
                               trace=trace)
    _CACHE["last_result"] = res

    bo = np.asarray(inputs["bo"], np.float32)
    out = np.empty((B, S, D), np.float32)
    for b in range(B):
        out[b] = (res.results[2 * b]["part"]
                  + res.results[2 * b + 1]["part"] + bo)
    return out



# revision 7
# speedup vs baseline: 1.0878x; 1.0878x over previous
"""MultiHeadAttention Trainium2 Bass kernel (8 NeuronCores), v2.

Reference computes (per batch b):
  qp = q @ Wq.T + bq            [S, H*D]   (S=2048, H=8, D=256)
  q_h = qp.reshape(H, S, D)     -- RAW reshape, not split-heads:
        head h <- qp rows [h*256,(h+1)*256), all 2048 cols;
        within head: s2 = ls*8 + g , d  <-> qp[h*256+ls, g*256+d]
  scores_h = q_h @ k_h.T / 16 ; P = softmax ; o_h = P @ v_h
  out[s2, h*256+d] = o_h[s2, d] ;  y = out @ Wo.T + bo

Sharding: core c = (b = c//2, hg = c%2) handles batch b, heads
hg*4..hg*4+4. Within a head we work in the permuted sequence order
s2' = g*256 + ls (softmax is row-wise so a consistent permutation of
rows/cols is exact); host applies the inverse permutation.

v2 vs baseline:
- All inputs bf16 (halves DMA); Q/K projections are quantized to
  fp8e4m3 (x A=8) so QK^T runs as ONE DoubleRow matmul per key chunk
  (contraction 256 = 2x128 folded, 0.5 cyc/row) instead of two f32r
  matmuls. exp() un-scales by 1/(16*A^2).
- P tiles bf16; PV + rowsum (ones-vector denominator) matmuls bf16.
- Output projection is Wo-stationary: yp[o, q] = sum_d WoT[d, o-chunk]
  . o_norm[d, q], so only 4 weight loads per head and N=512 moving
  operands. yacc accumulates [o x q']; host un-permutes/transposes.
- Emission interleaves QK(ig) with PV(ig-1) per key-chunk so the PE is
  never throttled to the exp/activation drain rate, and the next
  head's K/Q projections fill the PV-only pipeline steps (keeps HAM
  warm across head boundaries).
- PSUM: S2 (scores) + A2 (proj q/v + rowsum) + O2 (PV acc) + Y2
  (outproj + proj k) = 8 banks.
"""

import os as _os
import numpy as np

B, S, D, H = 4, 2048, 256, 8
HG = 2            # head groups (cores per batch)
HPG = H // HG     # heads per group = 4
SH = S // H       # seq rows owned by one head = 256
NCORES = 8
QA = 8.0          # fp8 quantization scale for q/k projections
EXP_SCALE = 1.0 / (16.0 * QA * QA)   # undo QA^2, apply 1/sqrt(d_k)

_CACHE = {}


def _build():
    import concourse.bacc as bacc
    import concourse.mybir as mybir
    from concourse.tile import TileContext

    F32 = mybir.dt.float32
    BF16 = mybir.dt.bfloat16
    F8 = mybir.dt.float8e4
    DR = mybir.MatmulPerfMode.DoubleRow
    EXP = mybir.ActivationFunctionType.Exp
    MULT = mybir.AluOpType.mult
    ADD = mybir.AluOpType.add

    nc = bacc.Bacc("TRN2", target_bir_lowering=False)

    SC = HPG * SH  # 1024 seq rows owned by this core

    # ---- DRAM I/O (per-core SPMD) ----
    xqT_d = nc.dram_tensor("xqT", [D, SC], BF16, kind="ExternalInput")
    xkT_d = nc.dram_tensor("xkT", [D, SC], BF16, kind="ExternalInput")
    xvT_d = nc.dram_tensor("xvT", [D, SC], BF16, kind="ExternalInput")
    WqT_d = nc.dram_tensor("WqT", [D, S], BF16, kind="ExternalInput")
    WkT_d = nc.dram_tensor("WkT", [D, S], BF16, kind="ExternalInput")
    WvT_d = nc.dram_tensor("WvT", [D, S], BF16, kind="ExternalInput")
    WoT_d = nc.dram_tensor("WoT", [HPG * D, D], BF16, kind="ExternalInput")
    bqT_d = nc.dram_tensor("bqT", [128, 16], F32, kind="ExternalInput")
    bkT_d = nc.dram_tensor("bkT", [128, 16], F32, kind="ExternalInput")
    bvr_d = nc.dram_tensor("bvr", [1, S], F32, kind="ExternalInput")
    out_d = nc.dram_tensor("part", [D, S], F32, kind="ExternalOutput")

    with TileContext(nc) as tc:
        with nc.allow_low_precision(reason="bf16/fp8 attention"), \
             tc.tile_pool(name="sb", bufs=1) as sb, \
             tc.tile_pool(name="ps", bufs=1, space="PSUM") as ps:

            def sbt(shape, dt, tag, **kw):
                return sb.tile(shape, dt, tag=tag, name=tag, **kw)

            # ---- persistent SBUF tiles ----
            WqT = [sbt([128, S], BF16, f"wq{i}") for i in range(2)]
            WkT = [sbt([128, S], BF16, f"wk{i}") for i in range(2)]
            WvT = [sbt([128, S], BF16, f"wv{i}") for i in range(2)]
            xqT = [sbt([128, SC], BF16, f"xq{i}") for i in range(2)]
            xkT = [sbt([128, SC], BF16, f"xk{i}") for i in range(2)]
            xvT = [sbt([128, SC], BF16, f"xv{i}") for i in range(2)]
            WoT = [sbt([128, D], BF16, f"wo{i}") for i in range(8)]
            bqT = sbt([128, 16], F32, "bqT")
            bkT = sbt([128, 16], F32, "bkT")
            bvr = sbt([1, S], F32, "bvr")
            bvb = sbt([128, S], F32, "bvb")
            # per-head projection buffers (dedicated: no WAR pressure).
            # 2D tiles (the DVE 3D-slice write path corrupts data on HW);
            # DoubleRow matmuls read them through a rearranged 3D view.
            qf8 = [sbt([128, 2 * S], F8, f"qf8_{t}") for t in range(HPG)]
            kf8 = [sbt([128, 2 * S], F8, f"kf8_{t}") for t in range(HPG)]
            qf8_3 = [q[:].rearrange("p (c s) -> p c s", c=2) for q in qf8]
            kf8_3 = [k[:].rearrange("p (c s) -> p c s", c=2) for k in kf8]
            vproj = [[sbt([128, S], BF16, f"vp{t}_{sc}") for sc in range(2)]
                     for t in range(HPG)]
            o_sb = [sbt([128, S], BF16, f"osb{dc}") for dc in range(2)]
            yaccT = [sbt([128, S], F32, f"yacc{oc}") for oc in range(2)]

            # ---- startup DMAs: priority order, round-robin 2 queues ----
            qs = [nc.sync, nc.scalar]
            _qi = [0]

            def dma(dst, src):
                qs[_qi[0] % len(qs)].dma_start(dst, src)
                _qi[0] += 1

            dma(bqT[:], bqT_d[:])
            dma(bkT[:], bkT_d[:])
            dma(bvr[:], bvr_d[:])
            # Q path for head 0 first
            for i in range(2):
                dma(xqT[i][:, 0:SH], xqT_d[i * 128:(i + 1) * 128, 0:SH])
            for half in range(2):
                for i in range(2):
                    dma(WqT[i][:, half * 1024:(half + 1) * 1024],
                        WqT_d[i * 128:(i + 1) * 128, half * 1024:(half + 1) * 1024])
            # K path for head 0
            for i in range(2):
                dma(xkT[i][:, 0:SH], xkT_d[i * 128:(i + 1) * 128, 0:SH])
            for half in range(2):
                for i in range(2):
                    dma(WkT[i][:, half * 1024:(half + 1) * 1024],
                        WkT_d[i * 128:(i + 1) * 128, half * 1024:(half + 1) * 1024])
            # V path for head 0
            for i in range(2):
                dma(xvT[i][:, 0:SH], xvT_d[i * 128:(i + 1) * 128, 0:SH])
            for half in range(2):
                for i in range(2):
                    dma(WvT[i][:, half * 1024:(half + 1) * 1024],
                        WvT_d[i * 128:(i + 1) * 128, half * 1024:(half + 1) * 1024])
            # remaining x columns (heads 1-3), Wo
            for i in range(2):
                dma(xqT[i][:, SH:SC], xqT_d[i * 128:(i + 1) * 128, SH:SC])
                dma(xkT[i][:, SH:SC], xkT_d[i * 128:(i + 1) * 128, SH:SC])
                dma(xvT[i][:, SH:SC], xvT_d[i * 128:(i + 1) * 128, SH:SC])
            for i in range(8):
                dma(WoT[i][:], WoT_d[i * 128:(i + 1) * 128, :])

            nc.gpsimd.partition_broadcast(bvb[:], bvr[:])

            ones_f = sbt([128, 1], F32, "ones_f")
            nc.vector.memset(ones_f[:], 1.0)
            ones = sbt([128, 1], BF16, "ones")
            nc.vector.tensor_copy(ones[:], ones_f[:])

            # ---- emitters ----
            P = {}       # (ig, jc) -> pt tile (keyed per head transiently)
            STATE = {}   # ig -> (rs, o_ps pair)

            def emit_qchunk(t, ec):
                pq = ps.tile([128, 512], F32, tag="A", bufs=2, name="pq")
                for dc in range(2):
                    nc.tensor.matmul(
                        pq[:, :SH],
                        WqT[dc][:, ec * 128:(ec + 1) * 128],
                        xqT[dc][:, t * SH:(t + 1) * SH],
                        start=(dc == 0), stop=(dc == 1))
                g, dct = divmod(ec, 2)
                nc.vector.tensor_scalar(
                    out=qf8[t][:, dct * S + g * SH:dct * S + (g + 1) * SH],
                    in0=pq[:, :SH], scalar1=QA,
                    scalar2=bqT[:, ec:ec + 1], op0=MULT, op1=ADD)

            def emit_kchunk(t, ec):
                pk = ps.tile([128, 512], F32, tag="Y", bufs=2, name="pk")
                for dc in range(2):
                    nc.tensor.matmul(
                        pk[:, :SH],
                        WkT[dc][:, ec * 128:(ec + 1) * 128],
                        xkT[dc][:, t * SH:(t + 1) * SH],
                        start=(dc == 0), stop=(dc == 1))
                g, dct = divmod(ec, 2)
                nc.vector.tensor_scalar(
                    out=kf8[t][:, dct * S + g * SH:dct * S + (g + 1) * SH],
                    in0=pk[:, :SH], scalar1=QA,
                    scalar2=bkT[:, ec:ec + 1], op0=MULT, op1=ADD)

            def emit_vchunk(t, i):
                sc, ng = divmod(i, 4)
                pv = ps.tile([128, 512], F32, tag="A", bufs=2, name="pv")
                for dc in range(2):
                    nc.tensor.matmul(
                        pv[:],
                        xvT[dc][:, t * SH + sc * 128:t * SH + (sc + 1) * 128],
                        WvT[dc][:, ng * 512:(ng + 1) * 512],
                        start=(dc == 0), stop=(dc == 1))
                nc.vector.tensor_add(
                    vproj[t][sc][:, ng * 512:(ng + 1) * 512], pv[:],
                    bvb[:, ng * 512:(ng + 1) * 512])

            def emit_qk(h, ig, jc):
                sp = ps.tile([128, 512], F32, tag="S", bufs=2, name="sp")
                nc.tensor.matmul(
                    sp[:],
                    kf8_3[h][:, :, jc * 128:(jc + 1) * 128],
                    qf8_3[h][:, :, ig * 512:(ig + 1) * 512],
                    start=True, stop=True, perf_mode=DR)
                pt = sb.tile([128, 512], BF16, tag="p", bufs=32, name="pt")
                nc.scalar.activation(pt[:], sp[:], EXP, scale=EXP_SCALE)
                P[(ig, jc)] = pt

            def emit_pv(h, ig, jc):
                if jc == 0:
                    rs = ps.tile([128, 512], F32, tag="A", bufs=2, name="rs")
                    o_ps = [ps.tile([128, 512], F32, tag="O", bufs=2,
                                    name=f"o{dc}") for dc in range(2)]
                    STATE[ig] = (rs, o_ps)
                rs, o_ps = STATE[ig]
                g, half = divmod(jc, 2)
                pt = P.pop((ig, jc))
                nc.tensor.matmul(
                    rs[0:1, :], ones[:], pt[:],
                    start=(jc == 0), stop=(jc == 15), skip_group_check=True)
                for dc in range(2):
                    nc.tensor.matmul(
                        o_ps[dc][:],
                        vproj[h][half][:, g * SH + dc * 128:g * SH + (dc + 1) * 128],
                        pt[:],
                        start=(jc == 0), stop=(jc == 15), skip_group_check=True)

            def emit_norm(h, ig):
                rs, o_ps = STATE.pop(ig)
                rcp = sb.tile([1, 512], F32, tag="rcp", bufs=2, name="rcp")
                nc.vector.reciprocal(rcp[:], rs[0:1, :])
                bc = sb.tile([128, 512], F32, tag="bc", bufs=2, name="bc")
                nc.gpsimd.partition_broadcast(bc[:], rcp[:])
                for dc in range(2):
                    nc.vector.tensor_mul(
                        o_sb[dc][:, ig * 512:(ig + 1) * 512], o_ps[dc][:], bc[:])

            def emit_outproj(h, ig):
                icol = ig * 512
                for oc in range(2):
                    yp = ps.tile([128, 512], F32, tag="Y", bufs=2, name="yp")
                    for dc in range(2):
                        nc.tensor.matmul(
                            yp[:],
                            WoT[h * 2 + dc][:, oc * 128:(oc + 1) * 128],
                            o_sb[dc][:, icol:icol + 512],
                            start=(dc == 0), stop=(dc == 1))
                    if h == 0:
                        nc.vector.tensor_copy(yaccT[oc][:, icol:icol + 512], yp[:])
                    else:
                        nc.vector.tensor_add(
                            yaccT[oc][:, icol:icol + 512],
                            yaccT[oc][:, icol:icol + 512], yp[:])
                        if h == HPG - 1:
                            nc.sync.dma_start(
                                out_d[oc * 128:(oc + 1) * 128, icol:icol + 512],
                                yaccT[oc][:, icol:icol + 512])

            # ---- head-0 projections (DMA-gated warmup) ----
            for ec in range(4):
                emit_qchunk(0, ec)
            for ec in range(16):
                emit_kchunk(0, ec)
            for ec in range(4, 16):
                emit_qchunk(0, ec)

            # ---- pipelined head loop ----
            # step S0: QK(ig0) + V-proj fillers [+ outproj(h-1, 3)]
            # steps S1-3: QK(ig) interleaved with PV(ig-1) [+ outproj(ig-2)]
            # step S4: PV(ig3) + K-proj(h+1) fillers [+ outproj(ig2)]
            # step S5: Q-proj(h+1) + outproj(ig3)
            for h in range(HPG):
                # S0
                for jc in range(16):
                    if jc % 2 == 0 and jc < 16:
                        vi = jc // 2
                        emit_vchunk(h, vi)
                    emit_qk(h, 0, jc)
                # S1..S3
                for ig in range(1, 4):
                    for jc in range(16):
                        emit_qk(h, ig, jc)
                        if ig >= 2 and jc == 6:
                            emit_outproj(h, ig - 2)
                        emit_pv(h, ig - 1, jc)
                    emit_norm(h, ig - 1)
                # S4
                for jc in range(16):
                    if h < HPG - 1:
                        emit_kchunk(h + 1, jc)
                    if jc == 6:
                        emit_outproj(h, 2)
                    emit_pv(h, 3, jc)
                emit_norm(h, 3)
                # S5
                if h < HPG - 1:
                    for ec in range(16):
                        if ec == 4:
                            emit_outproj(h, 3)
                        emit_qchunk(h + 1, ec)
                else:
                    emit_outproj(h, 3)

    nc.finalize()
    return nc


def _get_nc():
    if "nc" not in _CACHE:
        _CACHE["nc"] = _build()
    return _CACHE["nc"]


def _prep_inputs(query, key, values, Wq, bq, Wk, bk, Wv, bv, Wo, bo):
    import ml_dtypes
    bf16 = ml_dtypes.bfloat16
    f32 = np.float32
    query = np.asarray(query, f32)
    key = np.asarray(key, f32)
    values = np.asarray(values, f32)
    WqT = np.ascontiguousarray(np.asarray(Wq, f32).T.astype(bf16))
    WkT = np.ascontiguousarray(np.asarray(Wk, f32).T.astype(bf16))
    WvT = np.ascontiguousarray(np.asarray(Wv, f32).T.astype(bf16))
    WoT = np.ascontiguousarray(np.asarray(Wo, f32).T.astype(bf16))
    bqT = np.ascontiguousarray(
        (np.asarray(bq, f32) * QA).reshape(16, 128).T)
    bkT = np.ascontiguousarray(
        (np.asarray(bk, f32) * QA).reshape(16, 128).T)
    bvr = np.ascontiguousarray(np.asarray(bv, f32).reshape(1, S))

    in_maps = []
    for c in range(NCORES):
        b, hg = divmod(c, HG)
        rows = slice(hg * HPG * SH, (hg + 1) * HPG * SH)
        in_maps.append({
            "xqT": np.ascontiguousarray(query[b, rows, :].T.astype(bf16)),
            "xkT": np.ascontiguousarray(key[b, rows, :].T.astype(bf16)),
            "xvT": np.ascontiguousarray(values[b, rows, :].T.astype(bf16)),
            "WqT": WqT, "WkT": WkT, "WvT": WvT,
            "WoT": np.ascontiguousarray(WoT[hg * HPG * D:(hg + 1) * HPG * D, :]),
            "bqT": bqT, "bkT": bkT, "bvr": bvr,
        })
    return in_maps


def _enable_tracing_shims():
    """Best-effort: make trace=True survivable in environments where the
    image's antenv lacks axon_hooks and artifact upload has no network."""
    import sys
    import types
    try:
        import antenv.axon_hooks  # noqa: F401
    except Exception:
        try:
            from trn_agent_boot.trn_boot import _ntff_profile_via_ctypes
            hook = _ntff_profile_via_ctypes("/opt/axon/libaxon_pjrt.so")
            mod = types.ModuleType("antenv.axon_hooks")
            mod.get_axon_ntff_profile_hook = lambda: hook
            mod.set_axon_ntff_profile_hook = lambda h: None
            sys.modules["antenv.axon_hooks"] = mod
            import antenv
            antenv.axon_hooks = mod
        except Exception:
            pass
    try:
        import concourse.bass_utils as bu
        from concourse._compat import FishPath
        FishPath.bucket_root()
    except Exception:
        try:
            bu.upload_artifacts = lambda tmpdir: f"local://{tmpdir}"
        except Exception:
            pass


def kernel(**inputs):
    import os
    from concourse.bass_utils import run_bass_kernel_spmd

    nc = _get_nc()
    in_maps = _prep_inputs(**inputs)
    trace = bool(int(os.environ.get("KERNEL_TRACE", "0")))
    if trace or os.environ.get("BASS_TRACE"):
        _enable_tracing_shims()
    res = run_bass_kernel_spmd(nc, in_maps, core_ids=list(range(NCORES)),
                               trace=trace)
    _CACHE["last_result"] = res

    bo = np.asarray(inputs["bo"], np.float32)
    out = np.empty((B, S, D), np.float32)
    for b in range(B):
        # part[o, q'] with q' = g*256 + ls ; true s2 = ls*8 + g
        p0 = res.results[2 * b]["part"].reshape(D, 8, SH)
        p1 = res.results[2 * b + 1]["part"].reshape(D, 8, SH)
        y = (p0 + p1).transpose(2, 1, 0).reshape(S, D)
        out[b] = y + bo
    return out
